# revision 3
# baseline (speedup 1.0000x reference)
"""PointerGenerator kernel for 8 TRN2 NeuronCores (vocab-sharded tensor parallel).

reference computation:
    W_out = tanh(W_emb @ W_proj)                      # [V, 3d]
    c_cat = concat([hd_t, ce_t, cd_t], axis=1)        # [b, 3d]
    p0    = softmax(W_out @ c_cat.T + b_out)          # [b, V]
    p1    = scatter(ae_t -> vocab slots of x ids)     # [b, V]
    p_u   = sigmoid(W_u @ c_cat.T + b_u)              # [b, V]
    out   = p_u * p1 + (1 - p_u) * p0

Sharding: vocab axis across 8 cores (6400 rows each, V padded 50257->51200).
Each core computes its vocab slice of everything; softmax needs one [32,1]
AllReduce of the per-shard exp sums (no max subtraction: logits are O(1); the
vocab padding is masked by a -30000 bias so exp underflows to 0).
The ae_t scatter is done on-device with indirect DMA into a zeroed internal
DRAM buffer; host pre-masks ids per shard and dedups (last write wins).
"""

import numpy as np

BS = 32
SRC = 400
V = 50257
D_EMB = 512
D3 = 1536
N_CORES = 8
VP = 51200
VS = VP // N_CORES  # 6400 per core
KK = D_EMB // 128  # 4 contraction chunks for W_emb @ W_proj
KJ = D3 // 128  # 12 contraction chunks for logits
NCALL = 18  # scatter capacity: 18*128 = 2304 entries per core
P1_ROWS = (BS + 1) * VS  # row 32*VS.. is the trash row for masked entries
PAD_BIAS = -30000.0

_CACHE: dict = {}


def _build_program(ncall: int):
    import concourse.bacc as bacc
    import concourse.mybir as mybir
    import concourse.tile as tile

    f32 = mybir.dt.float32
    bf16 = mybir.dt.bfloat16
    i32 = mybir.dt.int32
    ACT = mybir.ActivationFunctionType

    nc = bacc.Bacc(
        "TRN2",
        target_bir_lowering=False,
        debug=False,
        enable_asserts=False,
        num_devices=N_CORES,
    )

    wembT = nc.dram_tensor("wembT", [D_EMB, VS], f32, kind="ExternalInput")
    wuT = nc.dram_tensor("wuT", [D3, VS], f32, kind="ExternalInput")
    wproj = nc.dram_tensor("wproj", [D_EMB, D3], f32, kind="ExternalInput")
    ccatT = nc.dram_tensor("ccatT", [D3, BS], f32, kind="ExternalInput")
    b_out = nc.dram_tensor("b_out", [1, VS], f32, kind="ExternalInput")
    b_u = nc.dram_tensor("b_u", [1, VS], f32, kind="ExternalInput")
    scat_off = nc.dram_tensor("scat_off", [128, ncall], i32, kind="ExternalInput")
    scat_val = nc.dram_tensor("scat_val", [128, ncall], f32, kind="ExternalInput")
    out = nc.dram_tensor("out", [BS, VS], f32, kind="ExternalOutput")

    # v-blocks: 12 x 512 + 1 x 256
    blocks = [(i * 512, 512) for i in range(12)] + [(12 * 512, 256)]

    with tile.TileContext(nc) as tc:
        with (
            tc.tile_pool(name="const", bufs=1) as const,
            tc.tile_pool(name="persist", bufs=1) as persist,
            tc.tile_pool(name="wemb", bufs=2) as wembp,
            tc.tile_pool(name="wu", bufs=4) as wup,
            tc.tile_pool(name="wout", bufs=3) as woutp,
            tc.tile_pool(name="work", bufs=3) as work,
            tc.tile_pool(name="psT", bufs=2, space="PSUM") as psT,
            tc.tile_pool(name="psL", bufs=2, space="PSUM") as psL,
            tc.tile_pool(name="dram", bufs=1, space="DRAM") as dram,
        ):
            # ---- stage 0: constants + scatter ----
            wproj_sb = const.tile([128, KK, D3], bf16)
            nc.gpsimd.dma_start(
                wproj_sb[:], wproj.ap().rearrange("(k p) j -> p k j", p=128)
            )
            ccat_sb = const.tile([128, KJ, BS], bf16)
            nc.gpsimd.dma_start(
                ccat_sb[:], ccatT.ap().rearrange("(j p) b -> p j b", p=128)
            )
            bo_sb = const.tile([1, VS], bf16)
            nc.gpsimd.dma_start(bo_sb[:], b_out.ap())
            bu_sb = const.tile([1, VS], bf16)
            nc.gpsimd.dma_start(bu_sb[:], b_u.ap())
            ones_sb = const.tile([1, BS], bf16)
            nc.vector.memset(ones_sb[:], 1.0)

            so_sb = const.tile([128, ncall], i32)
            nc.sync.dma_start(so_sb[:], scat_off.ap())
            sv_sb = const.tile([128, ncall], f32)
            nc.sync.dma_start(sv_sb[:], scat_val.ap())

            # p1 flat [P1_ROWS, 1]; zero it, then scatter ae values in
            p1_dram = dram.tile([P1_ROWS, 1], f32)
            zeros_sb = const.tile([128, P1_ROWS // 128], f32)
            nc.vector.memset(zeros_sb[:], 0.0)
            p1_zero_view = p1_dram[:].rearrange(
                "(p c) one -> p (c one)", p=128
            )
            nc.sync.dma_start(p1_zero_view, zeros_sb[:])
            import concourse.bass as bass

            for i in range(ncall):
                nc.gpsimd.indirect_dma_start(
                    out=p1_dram[:],
                    out_offset=bass.IndirectOffsetOnAxis(
                        ap=so_sb[:, i : i + 1], axis=0
                    ),
                    in_=sv_sb[:, i : i + 1],
                    in_offset=None,
                )
            # [b, v] view of p1 for the combine phase
            p1_bv = p1_dram[:].rearrange("(b v) one -> b (v one)", v=VS)

            out_sb = persist.tile([BS, VS], f32)  # accumulates p_u * p1
            bterm_sb = persist.tile([BS, VS], f32)  # (1 - p_u) * exp(logits)
            sums = persist.tile([BS, 16], f32)

            # ---- stage 1: main streaming loop over vocab blocks ----
            for blk, (vlo, n) in enumerate(blocks):
                wemb_t = wembp.tile([128, KK, 512], bf16, tag="wemb")
                for k in range(KK):
                    nc.gpsimd.dma_start(
                        wemb_t[:, k, :n],
                        wembT.ap()[k * 128 : (k + 1) * 128, vlo : vlo + n],
                    )

                l0_ps = psL.tile([BS, 512], f32, tag="l0")
                l1_ps = psL.tile([BS, 512], f32, tag="l1")

                for j in range(KJ):
                    # phase A: T^T[j-block, v-block] = sum_k Wproj[k,j]^T Wemb^T[k,v]
                    t_ps = psT.tile([128, 512], f32, tag="T")
                    for k in range(KK):
                        nc.tensor.matmul(
                            out=t_ps[:, :n],
                            lhsT=wproj_sb[:, k, j * 128 : (j + 1) * 128],
                            rhs=wemb_t[:, k, :n],
                            start=(k == 0),
                            stop=(k == KK - 1),
                        )
                    wout_t = woutp.tile([128, 512], bf16, tag="wout")
                    nc.scalar.activation(wout_t[:, :n], t_ps[:, :n], ACT.Tanh)

                    # p0 logits: accumulate c_catT[j]^T @ W_outT[j]
                    nc.tensor.matmul(
                        out=l0_ps[:, :n],
                        lhsT=ccat_sb[:, j, :],
                        rhs=wout_t[:, :n],
                        start=(j == 0),
                        stop=False,
                    )

                    # p_u logits: accumulate c_catT[j]^T @ W_uT[j]
                    wu_t = wup.tile([128, 512], bf16, tag="wu")
                    nc.gpsimd.dma_start(
                        wu_t[:, :n],
                        wuT.ap()[j * 128 : (j + 1) * 128, vlo : vlo + n],
                    )
                    nc.tensor.matmul(
                        out=l1_ps[:, :n],
                        lhsT=ccat_sb[:, j, :],
                        rhs=wu_t[:, :n],
                        start=(j == 0),
                        stop=False,
                    )

                # bias via K=1 matmul folded into the PSUM accumulation
                nc.tensor.matmul(
                    out=l0_ps[:, :n],
                    lhsT=ones_sb[:],
                    rhs=bo_sb[0:1, vlo : vlo + n],
                    start=False,
                    stop=True,
                )
                nc.tensor.matmul(
                    out=l1_ps[:, :n],
                    lhsT=ones_sb[:],
                    rhs=bu_sb[0:1, vlo : vlo + n],
                    start=False,
                    stop=True,
                )

                # epilogue: exp (+ partial sum), sigmoid, combine with p1
                sexp = work.tile([BS, 512], f32, tag="sexp")
                nc.scalar.activation(
                    sexp[:, :n],
                    l0_ps[:, :n],
                    ACT.Exp,
                    accum_out=sums[:, blk : blk + 1],
                )
                pu = work.tile([BS, 512], f32, tag="pu")
                nc.scalar.activation(pu[:, :n], l1_ps[:, :n], ACT.Sigmoid)

                p1_t = work.tile([BS, 512], f32, tag="p1")
                nc.sync.dma_start(p1_t[:, :n], p1_bv[0:BS, vlo : vlo + n])

                nc.vector.tensor_mul(
                    out_sb[:, vlo : vlo + n], pu[:, :n], p1_t[:, :n]
                )
                pusexp = work.tile([BS, 512], f32, tag="pus")
                nc.vector.tensor_mul(pusexp[:, :n], pu[:, :n], sexp[:, :n])
                nc.vector.tensor_sub(
                    bterm_sb[:, vlo : vlo + n], sexp[:, :n], pusexp[:, :n]
                )

            # ---- stage 2: global softmax denominator via AllReduce ----
            s_loc = persist.tile([BS, 1], f32)
            nc.vector.tensor_reduce(
                out=s_loc[:],
                in_=sums[:, : len(blocks)],
                axis=mybir.AxisListType.X,
                op=mybir.AluOpType.add,
            )
            cc_in = dram.tile([BS, 1], f32)
            cc_out = dram.tile([BS, 1], f32)
            nc.gpsimd.dma_start(cc_in[:], s_loc[:])
            nc.gpsimd.collective_compute(
                "AllReduce",
                mybir.AluOpType.add,
                replica_groups=[list(range(N_CORES))],
                ins=[cc_in.opt()],
                outs=[cc_out.opt()],
            )
            s_glob = persist.tile([BS, 1], f32)
            nc.gpsimd.dma_start(s_glob[:], cc_out[:])
            inv_s = persist.tile([BS, 1], f32)
            nc.vector.reciprocal(inv_s[:], s_glob[:])

            # ---- stage 3: out = p_u*p1 + (1-p_u)*exp * (1/S) ----
            nc.vector.tensor_scalar_mul(bterm_sb[:], bterm_sb[:], inv_s[:, 0:1])
            nc.vector.tensor_add(out_sb[:], out_sb[:], bterm_sb[:])
            nc.sync.dma_start(out.ap(), out_sb[:])

    nc.compile()
    return nc


def _get_program(ncall: int):
    if ncall not in _CACHE:
        _CACHE[ncall] = _build_program(ncall)
    return _CACHE[ncall]


def _prep_inputs(x, ae_t, ce_t, hd_t, cd_t, W_emb, W_proj, b_out, W_u, b_u):
    ids = np.asarray(x).reshape(BS, SRC).astype(np.int64)
    ae = np.ascontiguousarray(np.asarray(ae_t, dtype=np.float32))

    wembT = np.zeros((D_EMB, VP), np.float32)
    wembT[:, :V] = np.asarray(W_emb, dtype=np.float32).T
    wuT = np.zeros((D3, VP), np.float32)
    wuT[:, :V] = np.asarray(W_u, dtype=np.float32).T
    bo = np.full((VP,), PAD_BIAS, np.float32)
    bo[:V] = np.asarray(b_out, dtype=np.float32)
    bu = np.zeros((VP,), np.float32)
    bu[:V] = np.asarray(b_u, dtype=np.float32)
    ccT = np.ascontiguousarray(
        np.concatenate(
            [np.asarray(hd_t), np.asarray(ce_t), np.asarray(cd_t)], axis=1
        ).T.astype(np.float32)
    )
    wproj = np.ascontiguousarray(np.asarray(W_proj, dtype=np.float32))

    # per-core scatter lists: mask ids to the shard, dedup keeping the LAST
    # (b, s) occurrence (matches XLA scatter-set update order)
    per_core_scat = []
    for c in range(N_CORES):
        lo = c * VS
        sel = (ids >= lo) & (ids < lo + VS)
        b_idx, s_idx = np.nonzero(sel)  # row-major: s ascending within each b
        loc = (ids[b_idx, s_idx] - lo) + b_idx * VS
        vals = ae[b_idx, s_idx]
        # keep last occurrence of each loc
        _, first_rev = np.unique(loc[::-1], return_index=True)
        keep = len(loc) - 1 - first_rev
        per_core_scat.append((loc[keep], vals[keep]))

    n_max = max(len(l) for l, _ in per_core_scat)
    ncall = max(NCALL, -(-n_max // 128))
    cap = ncall * 128

    in_maps = []
    for c in range(N_CORES):
        loc, vals = per_core_scat[c]
        off_pad = np.full((cap,), BS * VS, np.int32)  # trash row
        val_pad = np.zeros((cap,), np.float32)
        off_pad[: len(loc)] = loc.astype(np.int32)
        val_pad[: len(loc)] = vals
        in_maps.append(
            {
                "wembT": np.ascontiguousarray(wembT[:, c * VS : (c + 1) * VS]),
                "wuT": np.ascontiguousarray(wuT[:, c * VS : (c + 1) * VS]),
                "wproj": wproj,
                "ccatT": ccT,
                "b_out": bo[c * VS : (c + 1) * VS].reshape(1, VS),
                "b_u": bu[c * VS : (c + 1) * VS].reshape(1, VS),
                "scat_off": off_pad.reshape(ncall, 128).T.copy(),
                "scat_val": val_pad.reshape(ncall, 128).T.copy(),
            }
        )
    return in_maps, ncall


def kernel(**inputs) -> np.ndarray:
    from concourse import bass_utils

    in_maps, ncall = _prep_inputs(**inputs)
    nc = _get_program(ncall)
    res = bass_utils.run_bass_kernel_spmd(
        nc, in_maps, core_ids=list(range(N_CORES))
    )
    full = np.concatenate([res.results[c]["out"] for c in range(N_CORES)], axis=1)
    return np.ascontiguousarray(full[:, :V])


# revision 8
# speedup vs baseline: 1.3714x; 1.3714x over previous
"""PointerGenerator kernel for 8 TRN2 NeuronCores (vocab-sharded tensor parallel).

reference computation:
    W_out = tanh(W_emb @ W_proj)                      # [V, 3d]
    c_cat = concat([hd_t, ce_t, cd_t], axis=1)        # [b, 3d]
    p0    = softmax(W_out @ c_cat.T + b_out)          # [b, V]
    p1    = scatter(ae_t -> vocab slots of x ids)     # [b, V]
    p_u   = sigmoid(W_u @ c_cat.T + b_u)              # [b, V]
    out   = p_u * p1 + (1 - p_u) * p0

Sharding: vocab axis across 8 cores (6400 true rows each; per-core width is
padded to 6656 = 13 blocks of 512 for uniform tiling). Each core computes its
vocab slice of everything; softmax needs one [32,1] AllReduce of the
per-shard exp sums (no max subtraction: logits are O(1); vocab padding is
masked via a -30000 bias so exp underflows to 0).
The ae_t scatter runs on-device via indirect DMA into a zeroed internal DRAM
buffer, concurrently with the main loop (p1 is only consumed at the end).
Host pre-masks ids per shard, dedups (last write wins), and pre-tiles the
bf16 weights so every stream DMA is one contiguous chunk.
The logits matmuls (M=32) use 4-way PE column tiling: 4 concurrent matmuls
in distinct 32-column groups of the array, folded by DVE adds afterwards.
"""

import numpy as np

BS = 32
SRC = 400
V = 50257
D_EMB = 512
D3 = 1536
N_CORES = 8
VS = 6400  # true vocab rows per core (8 * 6400 = 51200 >= V)
NB = 13  # uniform blocks per core
VN = 512  # block width
VST = NB * VN  # 6656 tiled per-core width (last 256 cols are padding)
KK = D_EMB // 128  # 4 contraction chunks for W_emb @ W_proj
KJ = D3 // 128  # 12 contraction chunks for logits
P1_ROWS = (BS + 1) * VST  # row BS*VST is the trash row for padded entries
PAD_BIAS = -30000.0

_CACHE: dict = {}


def _build_program(
    ncall: int,
    n_cores: int = N_CORES,
    collective: bool = True,
    repeat: int = 1,
    coltile: bool = True,
):
    import concourse.bacc as bacc
    import concourse.bass as bass
    import concourse.mybir as mybir
    import concourse.tile as tile

    f32 = mybir.dt.float32
    bf16 = mybir.dt.bfloat16
    i32 = mybir.dt.int32
    ACT = mybir.ActivationFunctionType

    nc = bacc.Bacc(
        "TRN2",
        target_bir_lowering=False,
        debug=False,
        enable_asserts=False,
        num_devices=n_cores,
    )

    # pre-tiled weights: one contiguous chunk per block / (block, j-group)
    wemb4 = nc.dram_tensor("wemb4", [NB, 128, KK * VN], bf16, kind="ExternalInput")
    wu12 = nc.dram_tensor("wu12", [NB, 128, KJ * VN], bf16, kind="ExternalInput")
    wproj = nc.dram_tensor("wproj", [D_EMB, D3], bf16, kind="ExternalInput")
    ccatT = nc.dram_tensor("ccatT", [D3, BS], bf16, kind="ExternalInput")
    b_out = nc.dram_tensor("b_out", [1, VST], bf16, kind="ExternalInput")
    b_u = nc.dram_tensor("b_u", [1, VST], bf16, kind="ExternalInput")
    scat_off = nc.dram_tensor("scat_off", [128, ncall], i32, kind="ExternalInput")
    scat_val = nc.dram_tensor("scat_val", [128, ncall], f32, kind="ExternalInput")
    out = nc.dram_tensor("out", [BS, VST], f32, kind="ExternalOutput")

    with tile.TileContext(nc) as tc:
        with (
            tc.tile_pool(name="const", bufs=1) as const,
            tc.tile_pool(name="persist", bufs=1) as persist,
            tc.tile_pool(name="wemb", bufs=2) as wembp,
            tc.tile_pool(name="wu", bufs=2) as wup,
            tc.tile_pool(name="wout", bufs=3) as woutp,
            tc.tile_pool(name="work", bufs=3) as work,
            tc.tile_pool(name="psT", bufs=2, space="PSUM") as psT,
            tc.tile_pool(name="psL", bufs=2, space="PSUM") as psL,
            tc.tile_pool(name="dram", bufs=1, space="DRAM") as dram,
        ):
            # ---- stage 0: constants ----
            wproj_sb = const.tile([128, KK, D3], bf16)
            nc.sync.dma_start(
                wproj_sb[:], wproj.ap().rearrange("(k p) j -> p k j", p=128)
            )
            ccat_sb = const.tile([128, KJ, BS], bf16)
            nc.sync.dma_start(
                ccat_sb[:], ccatT.ap().rearrange("(j p) b -> p j b", p=128)
            )
            bo_sb = const.tile([1, VST], bf16)
            nc.sync.dma_start(bo_sb[:], b_out.ap())
            bu_sb = const.tile([1, VST], bf16)
            nc.sync.dma_start(bu_sb[:], b_u.ap())
            ones_sb = const.tile([1, BS], bf16)
            nc.vector.memset(ones_sb[:], 1.0)

            # ---- scatter p1 (overlaps the main loop; consumed at the end) ----
            so_sb = const.tile([128, ncall], i32)
            nc.sync.dma_start(so_sb[:], scat_off.ap())
            sv_sb = const.tile([128, ncall], f32)
            nc.sync.dma_start(sv_sb[:], scat_val.ap())

            p1_dram = dram.tile([P1_ROWS, 1], f32)
            zeros_sb = const.tile([128, P1_ROWS // 128], f32)
            nc.vector.memset(zeros_sb[:], 0.0)
            nc.sync.dma_start(
                p1_dram[:].rearrange("(p c) one -> p (c one)", p=128), zeros_sb[:]
            )
            for i in range(ncall):
                nc.gpsimd.indirect_dma_start(
                    out=p1_dram[:],
                    out_offset=bass.IndirectOffsetOnAxis(
                        ap=so_sb[:, i : i + 1], axis=0
                    ),
                    in_=sv_sb[:, i : i + 1],
                    in_offset=None,
                )
            p1_bv = p1_dram[:].rearrange("(b v) one -> b (v one)", v=VST)

            pu_sb = persist.tile([BS, VST], f32)  # sigmoid(l_u)
            bterm_sb = persist.tile([BS, VST], f32)  # (1 - p_u) * exp(l0)
            sums = persist.tile([BS, 16], f32)

            def logits_matmuls(l0_ps, l1_ps, j, wout_t, wu_t):
                """Accumulate j-th contraction chunk into the logits PSUMs."""
                if coltile:
                    g = j % 4
                    nc.tensor.matmul(
                        out=l0_ps[32 * g : 32 * g + 32, :],
                        lhsT=ccat_sb[:, j, :],
                        rhs=wout_t[:],
                        start=(j < 4),
                        stop=False,
                        tile_position=(0, 32 * g),
                        skip_group_check=True,
                    )
                    nc.tensor.matmul(
                        out=l1_ps[32 * g : 32 * g + 32, :],
                        lhsT=ccat_sb[:, j, :],
                        rhs=wu_t[:, j, :],
                        start=(j < 4),
                        stop=False,
                        tile_position=(0, 32 * g),
                        skip_group_check=True,
                    )
                else:
                    nc.tensor.matmul(
                        out=l0_ps[:],
                        lhsT=ccat_sb[:, j, :],
                        rhs=wout_t[:],
                        start=(j == 0),
                        stop=False,
                    )
                    nc.tensor.matmul(
                        out=l1_ps[:],
                        lhsT=ccat_sb[:, j, :],
                        rhs=wu_t[:, j, :],
                        start=(j == 0),
                        stop=False,
                    )

            # ---- stage 1: main streaming loop over vocab blocks ----
            LP = 128 if coltile else BS  # logits psum partitions
            for _rep in range(repeat):
                for blk in range(NB):
                    vlo = blk * VN
                    wemb_t = wembp.tile([128, KK, VN], bf16, tag="wemb")
                    nc.sync.dma_start(
                        wemb_t[:],
                        wemb4.ap()[blk].rearrange("p (k v) -> p k v", k=KK),
                    )
                    wu_t = wup.tile([128, KJ, VN], bf16, tag="wu")
                    nc.sync.dma_start(
                        wu_t[:], wu12.ap()[blk].rearrange("p (j v) -> p j v", j=KJ)
                    )

                    l0_ps = psL.tile([LP, VN], f32, tag="l0")
                    l1_ps = psL.tile([LP, VN], f32, tag="l1")

                    for j in range(KJ):
                        # phase A: T^T[j, v] = sum_k Wproj[k,j]^T WembT[k,v]
                        t_ps = psT.tile([128, VN], f32, tag="T")
                        for k in range(KK):
                            nc.tensor.matmul(
                                out=t_ps[:],
                                lhsT=wproj_sb[:, k, j * 128 : (j + 1) * 128],
                                rhs=wemb_t[:, k, :],
                                start=(k == 0),
                                stop=(k == KK - 1),
                            )
                        wout_t = woutp.tile([128, VN], bf16, tag="wout")
                        nc.scalar.activation(wout_t[:], t_ps[:], ACT.Tanh)
                        logits_matmuls(l0_ps, l1_ps, j, wout_t, wu_t)

                    # bias via K=1 matmul folded into group 0's accumulation
                    nc.tensor.matmul(
                        out=l0_ps[0:BS, :],
                        lhsT=ones_sb[:],
                        rhs=bo_sb[0:1, vlo : vlo + VN],
                        start=False,
                        stop=True,
                        skip_group_check=True,
                    )
                    nc.tensor.matmul(
                        out=l1_ps[0:BS, :],
                        lhsT=ones_sb[:],
                        rhs=bu_sb[0:1, vlo : vlo + VN],
                        start=False,
                        stop=True,
                        skip_group_check=True,
                    )

                    if coltile:
                        # fold the 4 column groups: l = g0 + g1 + g2 + g3
                        l0_sb = work.tile([BS, VN], f32, tag="l0f")
                        l1_sb = work.tile([BS, VN], f32, tag="l1f")
                        for dst, src in ((l0_sb, l0_ps), (l1_sb, l1_ps)):
                            nc.vector.tensor_copy(dst[:], src[0:32, :])
                            nc.vector.tensor_add(dst[:], dst[:], src[32:64, :])
                            nc.vector.tensor_add(dst[:], dst[:], src[64:96, :])
                            nc.vector.tensor_add(dst[:], dst[:], src[96:128, :])
                        l0_v, l1_v = l0_sb, l1_sb
                    else:
                        l0_v, l1_v = l0_ps, l1_ps

                    # epilogue: exp (+ partial sum), sigmoids, (1-p_u)*exp
                    sexp = work.tile([BS, VN], f32, tag="sexp")
                    nc.scalar.activation(
                        sexp[:],
                        l0_v[:],
                        ACT.Exp,
                        accum_out=sums[:, blk : blk + 1],
                    )
                    nc.scalar.activation(
                        pu_sb[:, vlo : vlo + VN], l1_v[:], ACT.Sigmoid
                    )
                    om_pu = work.tile([BS, VN], f32, tag="ompu")
                    nc.scalar.activation(om_pu[:], l1_v[:], ACT.Sigmoid, scale=-1.0)
                    nc.vector.tensor_mul(
                        bterm_sb[:, vlo : vlo + VN], sexp[:], om_pu[:]
                    )

            # ---- stage 2: global softmax denominator via AllReduce ----
            s_loc = persist.tile([BS, 1], f32)
            nc.vector.tensor_reduce(
                out=s_loc[:],
                in_=sums[:, :NB],
                axis=mybir.AxisListType.X,
                op=mybir.AluOpType.add,
            )
            cc_in = dram.tile([BS, 1], f32)
            cc_out = dram.tile([BS, 1], f32)
            nc.gpsimd.dma_start(cc_in[:], s_loc[:])
            if collective:
                nc.gpsimd.collective_compute(
                    "AllReduce",
                    mybir.AluOpType.add,
                    replica_groups=[list(range(n_cores))],
                    ins=[cc_in.opt()],
                    outs=[cc_out.opt()],
                )
            else:
                nc.gpsimd.dma_start(cc_out[:], cc_in[:])
            s_glob = persist.tile([BS, 1], f32)
            nc.gpsimd.dma_start(s_glob[:], cc_out[:])
            inv_s = persist.tile([BS, 1], f32)
            nc.vector.reciprocal(inv_s[:], s_glob[:])

            # ---- stage 3: out = p_u*p1 + (1-p_u)*exp * (1/S) ----
            p1_full = persist.tile([BS, VST], f32)
            nc.sync.dma_start(p1_full[:], p1_bv[0:BS, :])
            nc.vector.tensor_scalar_mul(bterm_sb[:], bterm_sb[:], inv_s[:, 0:1])
            nc.vector.tensor_mul(pu_sb[:], pu_sb[:], p1_full[:])
            nc.vector.tensor_add(pu_sb[:], pu_sb[:], bterm_sb[:])
            nc.sync.dma_start(out.ap(), pu_sb[:])

    nc.compile()
    return nc


def _get_program(ncall: int):
    if ncall not in _CACHE:
        _CACHE[ncall] = _build_program(ncall)
    return _CACHE[ncall]


def _prep_inputs(x, ae_t, ce_t, hd_t, cd_t, W_emb, W_proj, b_out, W_u, b_u):
    import ml_dtypes

    bf = ml_dtypes.bfloat16
    ids = np.asarray(x).reshape(BS, SRC).astype(np.int64)
    ae = np.ascontiguousarray(np.asarray(ae_t, dtype=np.float32))

    # transposed weights in bf16, padded to the full sharded width
    VPAD = N_CORES * VS  # 51200
    wembT = np.zeros((D_EMB, VPAD), bf)
    wembT[:, :V] = np.asarray(W_emb, dtype=np.float32).T.astype(bf)
    wuT = np.zeros((D3, VPAD), bf)
    wuT[:, :V] = np.asarray(W_u, dtype=np.float32).T.astype(bf)
    bo = np.full((VPAD,), PAD_BIAS, bf)
    bo[:V] = np.asarray(b_out, dtype=np.float32).astype(bf)
    bu = np.zeros((VPAD,), bf)
    bu[:V] = np.asarray(b_u, dtype=np.float32).astype(bf)
    ccT = np.ascontiguousarray(
        np.concatenate(
            [np.asarray(hd_t), np.asarray(ce_t), np.asarray(cd_t)], axis=1
        ).T.astype(bf)
    )
    wproj_bf = np.ascontiguousarray(np.asarray(W_proj, dtype=np.float32).astype(bf))

    # per-core scatter lists: mask ids to the shard, dedup keeping the LAST
    # (b, s) occurrence (matches XLA scatter-set update order)
    per_core_scat = []
    for c in range(N_CORES):
        lo = c * VS
        sel = (ids >= lo) & (ids < lo + VS)
        b_idx, s_idx = np.nonzero(sel)  # row-major: s ascending within each b
        loc = (ids[b_idx, s_idx] - lo) + b_idx * VST
        vals = ae[b_idx, s_idx]
        _, first_rev = np.unique(loc[::-1], return_index=True)
        keep = len(loc) - 1 - first_rev
        per_core_scat.append((loc[keep], vals[keep]))

    n_max = max(len(l) for l, _ in per_core_scat)
    ncall = max(1, -(-n_max // 128))
    cap = ncall * 128

    in_maps = []
    for c in range(N_CORES):
        lo = c * VS
        loc, vals = per_core_scat[c]
        off_pad = np.full((cap,), BS * VST, np.int32)  # trash row
        val_pad = np.zeros((cap,), np.float32)
        off_pad[: len(loc)] = loc.astype(np.int32)
        val_pad[: len(loc)] = vals

        # per-core slices padded on the right from VS=6400 to VST=6656
        we = np.zeros((D_EMB, VST), bf)
        we[:, :VS] = wembT[:, lo : lo + VS]
        wu_c = np.zeros((D3, VST), bf)
        wu_c[:, :VS] = wuT[:, lo : lo + VS]
        bo_c = np.full((VST,), PAD_BIAS, bf)
        bo_c[:VS] = bo[lo : lo + VS]
        bu_c = np.zeros((VST,), bf)
        bu_c[:VS] = bu[lo : lo + VS]

        # pre-tile: [NB, 128, KK*VN] with [blk, p, k*VN+v] = we[k*128+p, blk*VN+v]
        we4 = np.ascontiguousarray(
            we.reshape(KK, 128, NB, VN).transpose(2, 1, 0, 3).reshape(NB, 128, KK * VN)
        )
        wu12 = np.ascontiguousarray(
            wu_c.reshape(KJ, 128, NB, VN)
            .transpose(2, 1, 0, 3)
            .reshape(NB, 128, KJ * VN)
        )
        in_maps.append(
            {
                "wemb4": we4,
                "wu12": wu12,
                "wproj": wproj_bf,
                "ccatT": ccT,
                "b_out": bo_c.reshape(1, VST),
                "b_u": bu_c.reshape(1, VST),
                "scat_off": off_pad.reshape(ncall, 128).T.copy(),
                "scat_val": val_pad.reshape(ncall, 128).T.copy(),
            }
        )
    return in_maps, ncall


def kernel(**inputs) -> np.ndarray:
    from concourse import bass_utils

    in_maps, ncall = _prep_inputs(**inputs)
    nc = _get_program(ncall)
    res = bass_utils.run_bass_kernel_spmd(nc, in_maps, core_ids=list(range(N_CORES)))
    full = np.concatenate(
        [res.results[c]["out"][:, :VS] for c in range(N_CORES)], axis=1
    )
    return np.ascontiguousarray(full[:, :V])
